# revision 1
# baseline (speedup 1.0000x reference)
"""Cross-attention (B=4, T=S=1024, C=1024, H=16, D=64) on 8 trn2 NeuronCores.

Sharding: core c handles batch b=c//2, sequence half hf=c%2 (512 q-rows).
k/v are computed for the core's own 512 encoder rows and exchanged within
the (2b, 2b+1) pair via AllGather. All activations are kept channel-major
("transposed", [C, T]-style) on chip so no transposes are ever needed; the
host transposes the per-core inputs/outputs (cheap numpy .T copies).

Per-core pipeline (everything fp32r on the PE, ~1e-4 matmul error):
  1. qT = (x Wq + bq)^T, kT likewise, v = enc Wv + bv (natural [s, c] layout,
     stored per-head padded [s, 16, 65] with a ones column at slot 64).
     RMSNorm+residual on q/k: per-token rsqrt(mean(q^2)) via ones-matmul
     column reduction + K=1 outer-product broadcast of (1 + scale*rr).
  2. Pairwise AllGather of kT [1024,512] and v_aug [512,1040].
  3. Per head h: scoresT[s-tile, t] = kh^T.T @ qh^T (K=64; head pairs run
     concurrently on PE row groups), exp on ACT (scale=1/8), then
     y_aug[65,512] = sum_s [v_h | 1].T @ exp  -- row 64 is the softmax
     denominator Z. rb = outer(1/16, 1/Z) via K=1 matmul; attn/16 = exp*rb
     accumulates into attn_mean; yT = 16 * y_aug[0:64] * rb.
  4. youtT = (yT Wp + bp)^T, DMA out; host transposes + reassembles.
"""

import numpy as np

import concourse.bacc as bacc
import concourse.mybir as mybir
import concourse.tile as tile
from concourse.bass_utils import run_bass_kernel_spmd

F32 = mybir.dt.float32
F32R = mybir.dt.float32r
AF = mybir.ActivationFunctionType
ALU = mybir.AluOpType

B, T, S, C, H = 4, 1024, 1024, 1024, 16
D = C // H            # 64
TN = 512              # per-core q rows / kv rows
KT = 8                # contraction tiles (C/128)
MT = 8                # output-channel tiles
ST = 8                # global s tiles (S/128)
DA = D + 1            # augmented head width (ones column at 64)
GROUPS = [[0, 1], [2, 3], [4, 5], [6, 7]]


def build():
    nc = bacc.Bacc("TRN2", target_bir_lowering=False, debug=False, num_devices=8)

    xT_d = nc.dram_tensor("xT", [C, TN], F32R, kind="ExternalInput")
    encT_d = nc.dram_tensor("encT", [C, TN], F32R, kind="ExternalInput")
    wq_d = nc.dram_tensor("wq", [C, C], F32R, kind="ExternalInput")
    wk_d = nc.dram_tensor("wk", [C, C], F32R, kind="ExternalInput")
    wv_d = nc.dram_tensor("wv", [C, C], F32R, kind="ExternalInput")
    wp_d = nc.dram_tensor("wp", [C, C], F32R, kind="ExternalInput")
    bq_d = nc.dram_tensor("bq", [128, MT], F32, kind="ExternalInput")
    bk_d = nc.dram_tensor("bk", [128, MT], F32, kind="ExternalInput")
    bp_d = nc.dram_tensor("bp", [128, MT], F32, kind="ExternalInput")
    bv_d = nc.dram_tensor("bv", [C], F32, kind="ExternalInput")
    qs_d = nc.dram_tensor("qs", [C], F32R, kind="ExternalInput")
    ks_d = nc.dram_tensor("ks", [C], F32R, kind="ExternalInput")

    yT_o = nc.dram_tensor("youtT", [C, TN], F32, kind="ExternalOutput")
    am_o = nc.dram_tensor("ameanT", [S, TN], F32, kind="ExternalOutput")

    KSZ = C * TN
    VSZ = TN * H * DA
    kv_bounce = nc.dram_tensor("kv_bounce", [KSZ + VSZ], F32R)
    kvg = nc.dram_tensor("kvg", [2, KSZ + VSZ], F32R)

    with tile.TileContext(nc) as tc:
        with (
            tc.tile_pool(name="const", bufs=1) as cst,
            tc.tile_pool(name="qt", bufs=1) as qt_pool,
            tc.tile_pool(name="acc", bufs=1) as acc_pool,
            tc.tile_pool(name="yt", bufs=1) as yt_pool,
        ):
            # ---- constants ----
            ones_col_f = cst.tile([128, 1], F32)
            nc.vector.memset(ones_col_f[:], 1.0)
            ones_col = cst.tile([128, 1], F32R)
            nc.vector.tensor_copy(ones_col[:], ones_col_f[:])
            inv16_row_f = cst.tile([1, 128], F32)
            nc.vector.memset(inv16_row_f[:], 1.0 / 16.0)
            inv16_row = cst.tile([1, 128], F32R)
            nc.vector.tensor_copy(inv16_row[:], inv16_row_f[:])
            ones_hf = cst.tile([128, H], F32)
            nc.vector.memset(ones_hf[:], 1.0)
            eps_t = cst.tile([1, 1], F32)
            nc.vector.memset(eps_t[:], 1e-6)

            bq_sb = cst.tile([128, MT], F32)
            bk_sb = cst.tile([128, MT], F32)
            bp_sb = cst.tile([128, MT], F32)
            nc.sync.dma_start(bq_sb[:], bq_d.ap())
            nc.sync.dma_start(bk_sb[:], bk_d.ap())
            nc.sync.dma_start(bp_sb[:], bp_d.ap())
            bv_bc = cst.tile([128, C], F32)
            nc.sync.dma_start(bv_bc[:], bv_d.ap().partition_broadcast(128))
            qs_sb = cst.tile([1, C], F32R)
            ks_sb = cst.tile([1, C], F32R)
            nc.sync.dma_start(qs_sb[:], qs_d.ap().unsqueeze(0))
            nc.sync.dma_start(ks_sb[:], ks_d.ap().unsqueeze(0))

            qT = [qt_pool.tile([128, TN], F32R, tag=f"qT{m}", name=f"qT{m}") for m in range(MT)]
            acc = [acc_pool.tile([128, TN], F32, tag=f"acc{j}", name=f"acc{j}") for j in range(ST)]
            yT = [yt_pool.tile([128, TN], F32R, tag=f"yT{k}", name=f"yT{k}") for k in range(KT)]

            # ================= phase 1: projections =================
            with (
                tc.tile_pool(name="p1_in", bufs=1) as p1_in,
                tc.tile_pool(name="p1_w", bufs=2) as p1_w,
                tc.tile_pool(name="p1_kv", bufs=1) as p1_kv,
                tc.tile_pool(name="p1_sq", bufs=2) as p1_sq,
                tc.tile_pool(name="p1_ps", bufs=2, space="PSUM") as p1_ps,
                tc.tile_pool(name="p1_ss", bufs=1, space="PSUM") as p1_ss,
                tc.tile_pool(name="p1_f", bufs=2, space="PSUM") as p1_f,
            ):
                xT = [p1_in.tile([128, TN], F32R, tag=f"xT{k}", name=f"xTs{k}") for k in range(KT)]
                eT = [p1_in.tile([128, TN], F32R, tag=f"eT{k}", name=f"eTs{k}") for k in range(KT)]
                for k in range(KT):
                    nc.sync.dma_start(xT[k][:], xT_d.ap()[k * 128 : (k + 1) * 128, :])
                    nc.sync.dma_start(eT[k][:], encT_d.ap()[k * 128 : (k + 1) * 128, :])

                def qk_projection(w_d, in_tiles, bias_sb, scale_sb, out_tiles):
                    w_sb = p1_w.tile([128, KT, C], F32R, tag="W")
                    for k in range(KT):
                        nc.sync.dma_start(
                            w_sb[:, k, :], w_d.ap()[k * 128 : (k + 1) * 128, :]
                        )
                    ssum = p1_ss.tile([1, TN], F32, tag="ssum")
                    for m in range(MT):
                        ps = p1_ps.tile([128, TN], F32, tag="proj")
                        for k in range(KT):
                            nc.tensor.matmul(
                                ps[:],
                                w_sb[:, k, m * 128 : (m + 1) * 128],
                                in_tiles[k][:],
                                start=(k == 0),
                                stop=(k == KT - 1),
                            )
                        # bias add (psum f32 -> sbuf f32r)
                        nc.vector.tensor_scalar_add(
                            out_tiles[m][:], ps[:], bias_sb[:, m : m + 1]
                        )
                        sq = p1_sq.tile([128, TN], F32R, tag="sq")
                        nc.scalar.activation(sq[:], out_tiles[m][:], AF.Square)
                        nc.tensor.matmul(
                            ssum[:],
                            ones_col[:],
                            sq[:],
                            start=(m == 0),
                            stop=(m == MT - 1),
                        )
                    # rr = 1/sqrt(ssum/C + eps)
                    rms = p1_sq.tile([1, TN], F32, tag="rms")
                    nc.scalar.activation(
                        rms[:], ssum[:], AF.Sqrt, scale=1.0 / C, bias=eps_t[:]
                    )
                    rr = p1_sq.tile([1, TN], F32R, tag="rr")
                    with nc.allow_low_precision(reason="rms rsqrt broadcast"):
                        nc.vector.reciprocal(rr[:], rms[:])
                    for m in range(MT):
                        fps = p1_f.tile([128, TN], F32, tag="fps")
                        nc.tensor.matmul(
                            fps[:],
                            scale_sb[:, m * 128 : (m + 1) * 128],
                            rr[:],
                            start=True,
                            stop=True,
                        )
                        f1 = p1_sq.tile([128, TN], F32R, tag="f1")
                        nc.scalar.activation(f1[:], fps[:], AF.Copy, bias=1.0)
                        nc.vector.tensor_mul(out_tiles[m][:], out_tiles[m][:], f1[:])

                # k projection -> kT tiles then bounce out (collective first!)
                kT = [p1_kv.tile([128, TN], F32R, tag=f"kT{m}", name=f"kTs{m}") for m in range(MT)]
                qk_projection(wk_d, eT, bk_sb, ks_sb, kT)
                for m in range(MT):
                    nc.sync.dma_start(
                        kv_bounce.ap()[m * 128 * TN : (m + 1) * 128 * TN].rearrange(
                            "(p t) -> p t", t=TN
                        ),
                        kT[m][:],
                    )

                # v projection: natural [s, c] layout, per-head padded + ones col
                v_loc = [
                    p1_kv.tile([128, H, DA], F32R, tag=f"vl{st}", name=f"vl{st}") for st in range(4)
                ]
                wv_sb = p1_w.tile([128, KT, C], F32R, tag="W")
                for k in range(KT):
                    nc.sync.dma_start(
                        wv_sb[:, k, :], wv_d.ap()[k * 128 : (k + 1) * 128, :]
                    )
                for st in range(4):
                    for jh in range(2):
                        ps = p1_ps.tile([128, TN], F32, tag="proj")
                        for k in range(KT):
                            nc.tensor.matmul(
                                ps[:],
                                eT[k][:, st * 128 : (st + 1) * 128],
                                wv_sb[:, k, jh * 512 : (jh + 1) * 512],
                                start=(k == 0),
                                stop=(k == KT - 1),
                            )
                        nc.vector.tensor_add(
                            v_loc[st][:, jh * 8 : (jh + 1) * 8, 0:D],
                            ps[:].rearrange("p (h d) -> p h d", h=8),
                            bv_bc[:, jh * 512 : (jh + 1) * 512].rearrange(
                                "p (h d) -> p h d", h=8
                            ),
                        )
                    nc.vector.tensor_copy(v_loc[st][:, :, D], ones_hf[:])
                    nc.sync.dma_start(
                        kv_bounce.ap()[
                            KSZ + st * 128 * H * DA : KSZ + (st + 1) * 128 * H * DA
                        ].rearrange("(p x) -> p x", x=H * DA),
                        v_loc[st][:].rearrange("p h d -> p (h d)"),
                    )

                # single fused collective for k+v, overlapped with q projection
                nc.gpsimd.collective_compute(
                    "AllGather",
                    ALU.bypass,
                    replica_groups=GROUPS,
                    ins=[kv_bounce.ap()],
                    outs=[kvg.ap()],
                )

                # q projection (runs on PE while the collective is in flight)
                qk_projection(wq_d, xT, bq_sb, qs_sb, qT)

            # ================= phase 3: attention =================
            with (
                tc.tile_pool(name="p3_k", bufs=1) as p3_k,
                tc.tile_pool(name="p3_v", bufs=1) as p3_v,
                tc.tile_pool(name="p3_e", bufs=2) as p3_e,
                tc.tile_pool(name="p3_rb", bufs=2) as p3_rb,
                tc.tile_pool(name="p3_sc", bufs=4, space="PSUM") as p3_sc,
                tc.tile_pool(name="p3_y", bufs=2, space="PSUM") as p3_y,
                tc.tile_pool(name="p3_rp", bufs=2, space="PSUM") as p3_rp,
            ):
                kTf = [p3_k.tile([128, 2, TN], F32R, tag=f"kTf{m}", name=f"kTf{m}") for m in range(MT)]
                for m in range(MT):
                    nc.sync.dma_start(
                        kTf[m][:],
                        kvg.ap()[:, m * 128 * TN : (m + 1) * 128 * TN]
                        .rearrange("g (p t) -> g p t", t=TN)
                        .transpose([1, 0, 2]),
                    )
                vf = [p3_v.tile([128, H, DA], F32R, tag=f"vf{j}", name=f"vf{j}") for j in range(ST)]
                for j in range(ST):
                    st = j % 4
                    nc.sync.dma_start(
                        vf[j][:],
                        kvg.ap()[
                            j // 4,
                            KSZ + st * 128 * H * DA : KSZ + (st + 1) * 128 * H * DA,
                        ].rearrange("(p h d) -> p h d", h=H, d=DA),
                    )

                for p in range(H // 2):
                    mt = p
                    heads = [(2 * p, 0), (2 * p + 1, 64)]
                    # interleaved K=64 score matmuls: bases 0/64 land on
                    # disjoint PE row groups and run concurrently
                    exps = {0: [], 64: []}
                    for j in range(ST):
                        for h, base in heads:
                            sc = p3_sc.tile([128, TN], F32, tag="sc")
                            nc.tensor.matmul(
                                sc[:],
                                kTf[mt][base : base + 64, j // 4,
                                        (j % 4) * 128 : (j % 4 + 1) * 128],
                                qT[mt][base : base + 64, :],
                                start=True,
                                stop=True,
                            )
                            ex = p3_e.tile([128, TN], F32R, tag=f"exp{base}_{j}")
                            nc.scalar.activation(
                                ex[:], sc[:], AF.Exp, scale=float(D) ** -0.5
                            )
                            exps[base].append(ex)
                    for h, base in heads:
                        y_ps = p3_y.tile([DA, TN], F32, tag="y")
                        for j in range(ST):
                            nc.tensor.matmul(
                                y_ps[:],
                                vf[j][:, h, :],
                                exps[base][j][:],
                                start=(j == 0),
                                stop=(j == ST - 1),
                            )
                        recip = p3_rb.tile([1, TN], F32R, tag="recip")
                        with nc.allow_low_precision(reason="softmax 1/Z broadcast"):
                            nc.vector.reciprocal(recip[:], y_ps[64:65, :])
                        rb_ps = p3_rp.tile([128, TN], F32, tag="rb")
                        nc.tensor.matmul(
                            rb_ps[:], inv16_row[:], recip[:], start=True, stop=True
                        )
                        # yT[c-tile mt, rows base:base+64] = 16 * y_aug * rb
                        rb_sb = p3_rb.tile([128, TN], F32, tag="rbsb")
                        nc.scalar.activation(rb_sb[:], rb_ps[:], AF.Copy)
                        nc.vector.scalar_tensor_tensor(
                            yT[mt][base : base + 64, :],
                            y_ps[0:64, :],
                            16.0,
                            rb_sb[0:64, :],
                            ALU.mult,
                            ALU.mult,
                        )
                        # attn_mean: acc[j] += exp[j] * rb  (rb = 1/(16 Z))
                        for j in range(ST):
                            if h == 0:
                                nc.vector.tensor_mul(
                                    acc[j][:], exps[base][j][:], rb_ps[:]
                                )
                            else:
                                nc.vector.tensor_mul(
                                    exps[base][j][:], exps[base][j][:], rb_ps[:]
                                )
                                nc.vector.tensor_add(
                                    acc[j][:], acc[j][:],
                                    exps[base][j][:].bitcast(F32),
                                )

            for j in range(ST):
                nc.sync.dma_start(am_o.ap()[j * 128 : (j + 1) * 128, :], acc[j][:])

            # ================= phase 4: output projection =================
            with (
                tc.tile_pool(name="p4_w", bufs=1) as p4_w,
                tc.tile_pool(name="p4_o", bufs=2) as p4_o,
                tc.tile_pool(name="p4_ps", bufs=2, space="PSUM") as p4_ps,
            ):
                wp_sb = p4_w.tile([128, KT, C], F32R, tag="Wp")
                for k in range(KT):
                    nc.sync.dma_start(
                        wp_sb[:, k, :], wp_d.ap()[k * 128 : (k + 1) * 128, :]
                    )
                for m in range(MT):
                    ps = p4_ps.tile([128, TN], F32, tag="yo")
                    for k in range(KT):
                        nc.tensor.matmul(
                            ps[:],
                            wp_sb[:, k, m * 128 : (m + 1) * 128],
                            yT[k][:],
                            start=(k == 0),
                            stop=(k == KT - 1),
                        )
                    yo = p4_o.tile([128, TN], F32, tag="yo_sb")
                    nc.vector.tensor_scalar_add(yo[:], ps[:], bp_sb[:, m : m + 1])
                    nc.sync.dma_start(yT_o.ap()[m * 128 : (m + 1) * 128, :], yo[:])

    nc.compile()
    return nc


_NC_CACHE = None


def _get_nc():
    global _NC_CACHE
    if _NC_CACHE is None:
        _NC_CACHE = build()
    return _NC_CACHE


def make_in_maps(x, encoder_output, Wq, bq, Wk, bk, Wv, bv, q_scale, k_scale,
                 Wp, bp):
    x = np.asarray(x, np.float32)
    enc = np.asarray(encoder_output, np.float32)
    Wq = np.ascontiguousarray(np.asarray(Wq, np.float32))
    Wk = np.ascontiguousarray(np.asarray(Wk, np.float32))
    Wv = np.ascontiguousarray(np.asarray(Wv, np.float32))
    Wp = np.ascontiguousarray(np.asarray(Wp, np.float32))
    bq_t = np.ascontiguousarray(np.asarray(bq, np.float32).reshape(MT, 128).T)
    bk_t = np.ascontiguousarray(np.asarray(bk, np.float32).reshape(MT, 128).T)
    bp_t = np.ascontiguousarray(np.asarray(bp, np.float32).reshape(MT, 128).T)
    bv = np.ascontiguousarray(np.asarray(bv, np.float32))
    qs = np.ascontiguousarray(np.asarray(q_scale, np.float32))
    ks = np.ascontiguousarray(np.asarray(k_scale, np.float32))

    in_maps = []
    for c in range(8):
        b, hf = c // 2, c % 2
        xT = np.ascontiguousarray(x[b, hf * TN : (hf + 1) * TN, :].T)
        encT = np.ascontiguousarray(enc[b, hf * TN : (hf + 1) * TN, :].T)
        in_maps.append(
            dict(xT=xT, encT=encT, wq=Wq, wk=Wk, wv=Wv, wp=Wp,
                 bq=bq_t, bk=bk_t, bp=bp_t, bv=bv, qs=qs, ks=ks)
        )

    return in_maps


def kernel(x, encoder_output, Wq, bq, Wk, bk, Wv, bv, q_scale, k_scale, Wp, bp,
           _trace=False):
    in_maps = make_in_maps(x, encoder_output, Wq, bq, Wk, bk, Wv, bv, q_scale,
                           k_scale, Wp, bp)
    nc = _get_nc()
    res = run_bass_kernel_spmd(nc, in_maps, core_ids=list(range(8)), trace=_trace)

    y = np.empty((B, T, C), np.float32)
    amean = np.empty((B, T, S), np.float32)
    for c in range(8):
        b, hf = c // 2, c % 2
        r = res.results[c]
        y[b, hf * TN : (hf + 1) * TN, :] = r["youtT"].T
        amean[b, hf * TN : (hf + 1) * TN, :] = r["ameanT"].T
    if _trace:
        kernel.last_exec_time_ns = res.exec_time_ns
        kernel.last_results = res
    return y, amean



# revision 4
# speedup vs baseline: 25005.2430x; 25005.2430x over previous
"""Cross-attention (B=4, T=S=1024, C=1024, H=16, D=64) on 8 trn2 NeuronCores.

v4: collective-free fp16 pipeline, wide ops, software-pipelined v projection.

Sharding: core c handles batch b=c//2, query half hf=c%2 (512 q rows); every
core recomputes full-S k/v for its batch (cheaper than a pairwise AllGather).

Schedule (PE program order == execution order per engine):
  k proj (wide N=1024) -> v proj heads 0-7 -> q proj -> head pairs 0..3 with
  v proj heads 8-15 interleaved (PE fills elementwise-bound pair time) ->
  pairs 4..7 -> attn_mean tail + output projection.

Per pair: both heads' scores into one wide PSUM tile -> ONE wide exp
(exp(score/8 - 10), fp16, bias cancels in softmax); y_aug via v_aug with a
16.0 denominator column so recip gives 1/(16Z) directly; rb broadcast by one
wide K=1 matmul + ACT copy; attn_mean acc[j] += exp*rb with wide mul/add pairs
split DVE (j 0-5) / GpSimd (j 6-7).
"""

import numpy as np

import concourse.bacc as bacc
import concourse.mybir as mybir
import concourse.tile as tile
from concourse.bass_utils import run_bass_kernel_spmd

F32 = mybir.dt.float32
F16 = mybir.dt.float16
AF = mybir.ActivationFunctionType
ALU = mybir.AluOpType

B, T, S, C, H = 4, 1024, 1024, 1024, 16
D = C // H            # 64
TN = 512              # per-core q rows
KT = 8                # contraction tiles (C/128)
MT = 8                # output-channel tiles
ST = 8                # s tiles (S/128)
DA = D + 1            # augmented head width (16.0 column at 64)
EB = -10.0            # exp bias: exp(score/8 + EB), cancels in softmax


def build(stt_pool=False, dve_j=6, qf1_dve=False):
    nc = bacc.Bacc("TRN2", target_bir_lowering=False, debug=False, num_devices=8)

    xT_d = nc.dram_tensor("xT", [C, TN], F16, kind="ExternalInput")
    encT_d = nc.dram_tensor("encT", [C, S], F16, kind="ExternalInput")
    wq_d = nc.dram_tensor("wq", [C, C], F16, kind="ExternalInput")
    wk_d = nc.dram_tensor("wk", [C, C], F16, kind="ExternalInput")
    wv_d = nc.dram_tensor("wv", [C, C], F16, kind="ExternalInput")
    wp_d = nc.dram_tensor("wp", [C, C], F16, kind="ExternalInput")
    bq_d = nc.dram_tensor("bq", [128, MT], F32, kind="ExternalInput")
    bk_d = nc.dram_tensor("bk", [128, MT], F32, kind="ExternalInput")
    bp_r_d = nc.dram_tensor("bp_r", [C], F16, kind="ExternalInput")
    bv_d = nc.dram_tensor("bv", [C], F16, kind="ExternalInput")
    qs_d = nc.dram_tensor("qs", [C], F16, kind="ExternalInput")
    ks_d = nc.dram_tensor("ks", [C], F16, kind="ExternalInput")

    yT_o = nc.dram_tensor("youtT", [C, TN], F16, kind="ExternalOutput")
    am_o = nc.dram_tensor("ameanT", [S, TN], F16, kind="ExternalOutput")

    def wide_load(w_sb, w_d, chunks=1):
        ap = w_d.ap().rearrange("(k p) c -> p k c", p=128)
        kc = KT // chunks
        for i in range(chunks):
            nc.sync.dma_start(
                w_sb[:, i * kc : (i + 1) * kc, :], ap[:, i * kc : (i + 1) * kc, :]
            )

    with tile.TileContext(nc) as tc:
        with (
            tc.tile_pool(name="const", bufs=1) as cst,
            tc.tile_pool(name="qt", bufs=1) as qt_pool,
            tc.tile_pool(name="kt", bufs=1) as kt_pool,
            tc.tile_pool(name="vt", bufs=1) as vt_pool,
            tc.tile_pool(name="acc", bufs=1) as acc_pool,
            tc.tile_pool(name="yt", bufs=1) as yt_pool,
            tc.tile_pool(name="w", bufs=2) as w_pool,
            tc.tile_pool(name="inx", bufs=1) as in_pool,
        ):
            # ---- first-issue DMAs: k-projection inputs (2 chunks each) ----
            wk_sb = w_pool.tile([128, KT, C], F16, tag="W", name="wk_sb")
            eT = in_pool.tile([128, KT, S], F16, name="eT")
            eap = encT_d.ap().rearrange("(k p) s -> p k s", p=128)
            nc.scalar.dma_start(eT[:, 0:4, :], eap[:, 0:4, :])
            wide_load(wk_sb, wk_d, chunks=2)
            nc.scalar.dma_start(eT[:, 4:8, :], eap[:, 4:8, :])

            # ---- constants ----
            ones_col_f = cst.tile([128, 1], F32)
            nc.vector.memset(ones_col_f[:], 1.0)
            ones_col = cst.tile([128, 1], F16)
            nc.vector.tensor_copy(ones_col[:], ones_col_f[:])
            ones_row_f = cst.tile([1, 512], F32)
            nc.vector.memset(ones_row_f[:], 1.0)
            ones_wrow = cst.tile([1, 512], F16)
            nc.vector.tensor_copy(ones_wrow[:], ones_row_f[:])
            ones_row = ones_wrow[:, 0:128]
            eps_t = cst.tile([1, 1], F32)
            nc.vector.memset(eps_t[:], 1e-6)
            ebias_t = cst.tile([128, 1], F32)
            nc.vector.memset(ebias_t[:], EB)

            bq_sb = cst.tile([128, MT], F32)
            bk_sb = cst.tile([128, MT], F32)
            nc.sync.dma_start(bk_sb[:], bk_d.ap())
            nc.sync.dma_start(bq_sb[:], bq_d.ap())
            bv_sb = cst.tile([1, C], F16)
            qs_sb = cst.tile([1, C], F16)
            ks_sb = cst.tile([1, C], F16)
            bp_row = cst.tile([1, C], F16)
            nc.sync.dma_start(bv_sb[:], bv_d.ap().unsqueeze(0))
            nc.sync.dma_start(ks_sb[:], ks_d.ap().unsqueeze(0))
            nc.sync.dma_start(qs_sb[:], qs_d.ap().unsqueeze(0))
            nc.sync.dma_start(bp_row[:], bp_r_d.ap().unsqueeze(0))

            xT = in_pool.tile([128, KT, TN], F16, name="xTs")
            nc.sync.dma_start(xT[:], xT_d.ap().rearrange("(k p) t -> p k t", p=128))

            qT = [qt_pool.tile([128, TN], F16, tag=f"qT{m}", name=f"qT{m}") for m in range(MT)]
            kT = [kt_pool.tile([128, S], F16, tag=f"kT{m}", name=f"kT{m}") for m in range(MT)]
            vf = [vt_pool.tile([128, H, DA], F16, tag=f"vf{j}", name=f"vf{j}") for j in range(ST)]
            # wide accumulators: cols 0:512 even heads, 512:1024 odd heads
            acc = [acc_pool.tile([128, 2 * TN], F16, tag=f"acc{j}", name=f"acc{j}") for j in range(ST)]
            yT = [yt_pool.tile([128, TN], F16, tag=f"yT{k}", name=f"yT{k}") for k in range(KT)]

            # y_aug denominator column: 16.0 so 1/y_aug[64] = 1/(16 Z)
            for j in range(ST):
                nc.vector.memset(vf[j][:, :, D], 16.0)

            # ============ k projection + RMS (full S, wide N=1024) ============
            with (
                tc.tile_pool(name="pk_sq", bufs=3) as pk_sq,
                tc.tile_pool(name="pk_ps", bufs=2, space="PSUM") as pk_ps,
                tc.tile_pool(name="pk_ss", bufs=1, space="PSUM") as pk_ss,
                tc.tile_pool(name="pk_f", bufs=1, space="PSUM") as pk_f,
            ):
                ssum = [pk_ss.tile([1, 512], F32, tag=f"ssum{sh}", name=f"kss{sh}")
                        for sh in range(2)]
                for m in range(MT):
                    ps = pk_ps.tile([128, S], F32, tag="proj")
                    for sh in range(2):
                        for k in range(KT):
                            nc.tensor.matmul(
                                ps[:, sh * 512 : (sh + 1) * 512],
                                wk_sb[:, k, m * 128 : (m + 1) * 128],
                                eT[:, k, sh * 512 : (sh + 1) * 512],
                                start=(k == 0),
                                stop=(k == KT - 1),
                            )
                    nc.vector.tensor_scalar_add(kT[m][:], ps[:], bk_sb[:, m : m + 1])
                    sq = pk_sq.tile([128, S], F16, tag="sq")
                    nc.vector.tensor_mul(sq[:], kT[m][:], kT[m][:])
                    for sh in range(2):
                        nc.tensor.matmul(
                            ssum[sh][:], ones_col[:],
                            sq[:, sh * 512 : (sh + 1) * 512],
                            start=(m == 0), stop=(m == MT - 1),
                        )
                krr = pk_sq.tile([1, S], F16, tag="krr", name="krr")
                for sh in range(2):
                    rms = pk_sq.tile([1, 512], F32, tag="rms")
                    nc.scalar.activation(
                        rms[:], ssum[sh][:], AF.Sqrt, scale=1.0 / C, bias=eps_t[:]
                    )
                    with nc.allow_low_precision(reason="rms rsqrt broadcast"):
                        nc.vector.reciprocal(krr[:, sh * 512 : (sh + 1) * 512], rms[:])
                for m in range(MT):
                    fps = pk_f.tile([128, S], F32, tag="fps")
                    for sh in range(2):
                        nc.tensor.matmul(
                            fps[:, sh * 512 : (sh + 1) * 512],
                            ks_sb[:, m * 128 : (m + 1) * 128],
                            krr[:, sh * 512 : (sh + 1) * 512],
                            start=True, stop=True,
                        )
                    f1 = pk_sq.tile([128, S], F16, tag="f1")
                    nc.scalar.activation(f1[:], fps[:], AF.Copy, bias=1.0)
                    nc.vector.tensor_mul(kT[m][:], kT[m][:], f1[:])

            # v projection for one head half (heads jh*8 .. jh*8+7), all s
            def v_proj(pv_ps, wv_sb, jh, sts):
                for st in sts:
                    ps = pv_ps.tile([128, 512], F32, tag="proj")
                    for k in range(KT):
                        nc.tensor.matmul(
                            ps[:],
                            eT[:, k, st * 128 : (st + 1) * 128],
                            wv_sb[:, k, jh * 512 : (jh + 1) * 512],
                            start=(k == 0),
                            stop=False,
                        )
                    nc.tensor.matmul(
                        ps[:], ones_row[:], bv_sb[:, jh * 512 : (jh + 1) * 512],
                        start=False, stop=True,
                    )
                    nc.scalar.activation(
                        vf[st][:, jh * 8 : (jh + 1) * 8, 0:D],
                        ps[:].rearrange("p (h d) -> p h d", h=8),
                        AF.Copy,
                    )

            # ============ v projection heads 0-7 ============
            wv_sb = w_pool.tile([128, KT, C], F16, tag="W", name="wv_sb")
            with (
                tc.tile_pool(name="pv0_ps", bufs=2, space="PSUM") as pv0_ps,
            ):
                wide_load(wv_sb, wv_d)
                v_proj(pv0_ps, wv_sb, 0, range(ST))

            # ============ q projection + RMS (TN rows) ============
            with (
                tc.tile_pool(name="pq_sq", bufs=3) as pq_sq,
                tc.tile_pool(name="pq_ps", bufs=2, space="PSUM") as pq_ps,
                tc.tile_pool(name="pq_ss", bufs=1, space="PSUM") as pq_ss,
            ):
                wq_sb = w_pool.tile([128, KT, C], F16, tag="W", name="wq_sb")
                wide_load(wq_sb, wq_d)
                qss = pq_ss.tile([1, TN], F32, tag="qss", name="qss")
                for m in range(MT):
                    ps = pq_ps.tile([128, TN], F32, tag="proj")
                    for k in range(KT):
                        nc.tensor.matmul(
                            ps[:],
                            wq_sb[:, k, m * 128 : (m + 1) * 128],
                            xT[:, k, :],
                            start=(k == 0),
                            stop=(k == KT - 1),
                        )
                    nc.vector.tensor_scalar_add(qT[m][:], ps[:], bq_sb[:, m : m + 1])
                    sq = pq_sq.tile([128, TN], F16, tag="sq")
                    nc.vector.tensor_mul(sq[:], qT[m][:], qT[m][:])
                    nc.tensor.matmul(
                        qss[:], ones_col[:], sq[:],
                        start=(m == 0), stop=(m == MT - 1),
                    )
                qrms = pq_sq.tile([1, TN], F32, tag="qrms", name="qrms")
                nc.scalar.activation(
                    qrms[:], qss[:], AF.Sqrt, scale=1.0 / C, bias=eps_t[:]
                )
                qrr = pq_sq.tile([1, TN], F16, tag="qrr", name="qrr")
                with nc.allow_low_precision(reason="rms rsqrt broadcast"):
                    nc.vector.reciprocal(qrr[:], qrms[:])
                for m in range(MT):
                    fps = pq_ps.tile([128, TN], F32, tag="proj")
                    nc.tensor.matmul(
                        fps[:], qs_sb[:, m * 128 : (m + 1) * 128], qrr[:],
                        start=True, stop=True,
                    )
                    f1 = pq_sq.tile([128, TN], F16, tag="qf1")
                    if qf1_dve:
                        nc.vector.tensor_scalar_add(f1[:], fps[:], 1.0)
                    else:
                        nc.scalar.activation(f1[:], fps[:], AF.Copy, bias=1.0)
                    nc.vector.tensor_mul(qT[m][:], qT[m][:], f1[:])

            # ---- attention (pairs 0-3 interleave v-proj heads 8-15) ----
            if True:
                with (
                    tc.tile_pool(name="p3_e", bufs=2) as p3_e,
                    tc.tile_pool(name="p3_rb", bufs=2) as p3_rb,
                    tc.tile_pool(name="p3_t", bufs=3) as p3_t,
                    tc.tile_pool(name="pv_ps", bufs=2, space="PSUM") as pv_ps,
                    tc.tile_pool(name="p3_sc", bufs=2, space="PSUM") as p3_sc,
                    tc.tile_pool(name="p3_y", bufs=2, space="PSUM") as p3_y,
                ):
                    wp_sb = w_pool.tile([128, KT, C], F16, tag="W", name="wp_sb")
                    wide_load(wp_sb, wp_d)

                    def attn_pair(p):
                        mt = p
                        heads = [(2 * p, 0), (2 * p + 1, 64)]
                        exps = []
                        for j in range(ST):
                            sc = p3_sc.tile([128, 2 * TN], F32, tag="sc")
                            for h, base in heads:
                                nc.tensor.matmul(
                                    sc[:, base * 8 : base * 8 + TN],
                                    kT[mt][base : base + 64, j * 128 : (j + 1) * 128],
                                    qT[mt][base : base + 64, :],
                                    start=True,
                                    stop=True,
                                )
                            ex = p3_e.tile([128, 2 * TN], F16, tag=f"exp{j}")
                            nc.scalar.activation(
                                ex[:], sc[:], AF.Exp, scale=float(D) ** -0.5,
                                bias=ebias_t[:],
                            )
                            exps.append(ex)
                        recip = p3_rb.tile([1, 2 * TN], F16, tag="recip")
                        rb_sb = p3_rb.tile([128, 2 * TN], F16, tag="rbsb")
                        y_pss = []
                        for h, base in heads:
                            y_ps = p3_y.tile([DA, TN], F32, tag="y")
                            y_pss.append(y_ps)
                            for j in range(ST):
                                nc.tensor.matmul(
                                    y_ps[:],
                                    vf[j][:, h, :],
                                    exps[j][:, base * 8 : base * 8 + TN],
                                    start=(j == 0),
                                    stop=(j == ST - 1),
                                )
                            # per-head rb = 1/(16 Z) broadcast: head A's chain
                            # completes while head B's y matmuls run
                            with nc.allow_low_precision(reason="softmax 1/Z bcast"):
                                nc.vector.reciprocal(
                                    recip[:, base * 8 : base * 8 + TN], y_ps[64:65, :]
                                )
                            rb_ps = p3_sc.tile([128, 2 * TN], F32, tag="sc")
                            nc.tensor.matmul(
                                rb_ps[:, 0:TN],
                                ones_row[:],
                                recip[:, base * 8 : base * 8 + TN],
                                start=True, stop=True,
                            )
                            nc.scalar.activation(
                                rb_sb[:, base * 8 : base * 8 + TN],
                                rb_ps[:, 0:TN], AF.Copy,
                            )
                        # yT rows = 16 * y_aug * rb
                        for (h, base), y_ps in zip(heads, y_pss):
                            (nc.gpsimd if stt_pool else nc.vector).scalar_tensor_tensor(
                                yT[mt][base : base + 64, :],
                                y_ps[0:64, :],
                                16.0,
                                rb_sb[0:64, base * 8 : base * 8 + TN],
                                ALU.mult,
                                ALU.mult,
                            )
                        # attn_mean: acc[j] += exp[j] * rb (wide, both heads)
                        for j in range(ST):
                            eng = nc.vector if j < dve_j else nc.gpsimd
                            if p == 0:
                                eng.tensor_mul(acc[j][:], exps[j][:], rb_sb[:])
                            else:
                                t = p3_t.tile([128, 2 * TN], F16, tag=f"t{j % 3}")
                                eng.tensor_mul(t[:], exps[j][:], rb_sb[:])
                                eng.tensor_add(acc[j][:], acc[j][:], t[:])

                    for p in range(4):
                        attn_pair(p)
                        v_proj(pv_ps, wv_sb, 1, range(2 * p, 2 * p + 2))
                    for p in range(4, 8):
                        attn_pair(p)

                    # output projection (only needs yT; emitted before the
                    # attn_mean epilogue so PE overlaps the elementwise tail;
                    # bias via K=1 ones matmul + ACT copy keeps DVE free)
                    for m in range(MT):
                        ps = pv_ps.tile([128, TN], F32, tag="proj")
                        for k in range(KT):
                            nc.tensor.matmul(
                                ps[:],
                                wp_sb[:, k, m * 128 : (m + 1) * 128],
                                yT[k][:],
                                start=(k == 0),
                                stop=False,
                            )
                        nc.tensor.matmul(
                            ps[:],
                            bp_row[:, m * 128 : (m + 1) * 128],
                            ones_wrow[:],
                            start=False,
                            stop=True,
                        )
                        yo = p3_t.tile([128, TN], F16, tag=f"yo{m % 2}")
                        nc.scalar.activation(yo[:], ps[:], AF.Copy)
                        nc.sync.dma_start(yT_o.ap()[m * 128 : (m + 1) * 128, :], yo[:])

                    for j in range(ST):
                        amo = p3_t.tile([128, TN], F16, tag=f"amo{j % 2}")
                        eng = nc.gpsimd if j % 2 else nc.vector
                        eng.tensor_add(amo[:], acc[j][:, 0:TN], acc[j][:, TN:])
                        nc.sync.dma_start(
                            am_o.ap()[j * 128 : (j + 1) * 128, :], amo[:]
                        )

    nc.compile()
    return nc


_NC_CACHE = None


def _get_nc():
    global _NC_CACHE
    if _NC_CACHE is None:
        _NC_CACHE = build()
    return _NC_CACHE


def make_in_maps(x, encoder_output, Wq, bq, Wk, bk, Wv, bv, q_scale, k_scale,
                 Wp, bp):
    f16 = np.float16
    x = np.asarray(x, np.float32)
    enc = np.asarray(encoder_output, np.float32)
    Wq = np.ascontiguousarray(np.asarray(Wq, f16))
    Wk = np.ascontiguousarray(np.asarray(Wk, f16))
    Wv = np.ascontiguousarray(np.asarray(Wv, f16))
    Wp = np.ascontiguousarray(np.asarray(Wp, f16))
    bq_t = np.ascontiguousarray(np.asarray(bq, np.float32).reshape(MT, 128).T)
    bk_t = np.ascontiguousarray(np.asarray(bk, np.float32).reshape(MT, 128).T)
    bp_r = np.ascontiguousarray(np.asarray(bp, f16))
    bv = np.ascontiguousarray(np.asarray(bv, f16))
    qs = np.ascontiguousarray(np.asarray(q_scale, f16))
    ks = np.ascontiguousarray(np.asarray(k_scale, f16))

    in_maps = []
    for c in range(8):
        b, hf = c // 2, c % 2
        xT = np.ascontiguousarray(x[b, hf * TN : (hf + 1) * TN, :].T.astype(f16))
        encT = np.ascontiguousarray(enc[b].T.astype(f16))
        in_maps.append(
            dict(xT=xT, encT=encT, wq=Wq, wk=Wk, wv=Wv, wp=Wp,
                 bq=bq_t, bk=bk_t, bp_r=bp_r, bv=bv, qs=qs, ks=ks)
        )
    return in_maps


def kernel(x, encoder_output, Wq, bq, Wk, bk, Wv, bv, q_scale, k_scale, Wp, bp,
           _trace=False):
    in_maps = make_in_maps(x, encoder_output, Wq, bq, Wk, bk, Wv, bv, q_scale,
                           k_scale, Wp, bp)
    nc = _get_nc()
    res = run_bass_kernel_spmd(nc, in_maps, core_ids=list(range(8)), trace=_trace)

    y = np.empty((B, T, C), np.float32)
    amean = np.empty((B, T, S), np.float32)
    for c in range(8):
        b, hf = c // 2, c % 2
        r = res.results[c]
        y[b, hf * TN : (hf + 1) * TN, :] = r["youtT"].T.astype(np.float32)
        amean[b, hf * TN : (hf + 1) * TN, :] = r["ameanT"].T.astype(np.float32)
    if _trace:
        kernel.last_exec_time_ns = res.exec_time_ns
        kernel.last_results = res
    return y, amean


# revision 5
# speedup vs baseline: 25017.4674x; 1.0005x over previous
"""Cross-attention (B=4, T=S=1024, C=1024, H=16, D=64) on 8 trn2 NeuronCores.

v4: collective-free fp16 pipeline, wide ops, software-pipelined v projection.

Sharding: core c handles batch b=c//2, query half hf=c%2 (512 q rows); every
core recomputes full-S k/v for its batch (cheaper than a pairwise AllGather).

Schedule (PE program order == execution order per engine):
  k proj (wide N=1024) -> v proj heads 0-7 -> q proj -> head pairs 0..3 with
  v proj heads 8-15 interleaved (PE fills elementwise-bound pair time) ->
  pairs 4..7 -> attn_mean tail + output projection.

Per pair: both heads' scores into one wide PSUM tile -> ONE wide exp
(exp(score/8 - 10), fp16, bias cancels in softmax); y_aug via v_aug with a
16.0 denominator column so recip gives 1/(16Z) directly; rb broadcast by one
wide K=1 matmul + ACT copy; attn_mean acc[j] += exp*rb with wide mul/add pairs
split DVE (j 0-5) / GpSimd (j 6-7).
"""

import numpy as np

import concourse.bacc as bacc
import concourse.mybir as mybir
import concourse.tile as tile
from concourse.bass_utils import run_bass_kernel_spmd

F32 = mybir.dt.float32
F16 = mybir.dt.float16
AF = mybir.ActivationFunctionType
ALU = mybir.AluOpType

B, T, S, C, H = 4, 1024, 1024, 1024, 16
D = C // H            # 64
TN = 512              # per-core q rows
KT = 8                # contraction tiles (C/128)
MT = 8                # output-channel tiles
ST = 8                # s tiles (S/128)
DA = D + 1            # augmented head width (16.0 column at 64)
EB = -10.0            # exp bias: exp(score/8 + EB), cancels in softmax


def build(stt_pool=False, dve_j=6, qf1_dve=False):
    nc = bacc.Bacc("TRN2", target_bir_lowering=False, debug=False, num_devices=8)

    xT_d = nc.dram_tensor("xT", [C, TN], F16, kind="ExternalInput")
    encT_d = nc.dram_tensor("encT", [C, S], F16, kind="ExternalInput")
    wq_d = nc.dram_tensor("wq", [C, C], F16, kind="ExternalInput")
    wk_d = nc.dram_tensor("wk", [C, C], F16, kind="ExternalInput")
    wv_d = nc.dram_tensor("wv", [C, C], F16, kind="ExternalInput")
    wp_d = nc.dram_tensor("wp", [C, C], F16, kind="ExternalInput")
    bq_d = nc.dram_tensor("bq", [128, MT], F32, kind="ExternalInput")
    bk_d = nc.dram_tensor("bk", [128, MT], F32, kind="ExternalInput")
    bp_r_d = nc.dram_tensor("bp_r", [C], F16, kind="ExternalInput")
    bv_d = nc.dram_tensor("bv", [C], F16, kind="ExternalInput")
    qs_d = nc.dram_tensor("qs", [C], F16, kind="ExternalInput")
    ks_d = nc.dram_tensor("ks", [C], F16, kind="ExternalInput")

    yT_o = nc.dram_tensor("youtT", [C, TN], F16, kind="ExternalOutput")
    am_o = nc.dram_tensor("ameanT", [S, TN], F16, kind="ExternalOutput")

    def wide_load(w_sb, w_d, chunks=1):
        ap = w_d.ap().rearrange("(k p) c -> p k c", p=128)
        kc = KT // chunks
        for i in range(chunks):
            nc.sync.dma_start(
                w_sb[:, i * kc : (i + 1) * kc, :], ap[:, i * kc : (i + 1) * kc, :]
            )

    with tile.TileContext(nc) as tc:
        with (
            tc.tile_pool(name="const", bufs=1) as cst,
            tc.tile_pool(name="qt", bufs=1) as qt_pool,
            tc.tile_pool(name="kt", bufs=1) as kt_pool,
            tc.tile_pool(name="vt", bufs=1) as vt_pool,
            tc.tile_pool(name="acc", bufs=1) as acc_pool,
            tc.tile_pool(name="yt", bufs=1) as yt_pool,
            tc.tile_pool(name="w", bufs=2) as w_pool,
            tc.tile_pool(name="inx", bufs=1) as in_pool,
        ):
            # ---- first-issue DMAs: k-projection inputs (2 chunks each) ----
            wk_sb = w_pool.tile([128, KT, C], F16, tag="W", name="wk_sb")
            eT = in_pool.tile([128, KT, S], F16, name="eT")
            eap = encT_d.ap().rearrange("(k p) s -> p k s", p=128)
            nc.scalar.dma_start(eT[:, 0:4, :], eap[:, 0:4, :])
            wide_load(wk_sb, wk_d, chunks=2)
            nc.scalar.dma_start(eT[:, 4:8, :], eap[:, 4:8, :])

            # ---- constants ----
            ones_col_f = cst.tile([128, 1], F32)
            nc.vector.memset(ones_col_f[:], 1.0)
            ones_col = cst.tile([128, 1], F16)
            nc.vector.tensor_copy(ones_col[:], ones_col_f[:])
            ones_row_f = cst.tile([1, 512], F32)
            nc.vector.memset(ones_row_f[:], 1.0)
            ones_wrow = cst.tile([1, 512], F16)
            nc.vector.tensor_copy(ones_wrow[:], ones_row_f[:])
            ones_row = ones_wrow[:, 0:128]
            eps_t = cst.tile([1, 1], F32)
            nc.vector.memset(eps_t[:], 1e-6)
            ebias_t = cst.tile([128, 1], F32)
            nc.vector.memset(ebias_t[:], EB)

            bq_sb = cst.tile([128, MT], F32)
            bk_sb = cst.tile([128, MT], F32)
            nc.sync.dma_start(bk_sb[:], bk_d.ap())
            nc.sync.dma_start(bq_sb[:], bq_d.ap())
            bv_sb = cst.tile([1, C], F16)
            qs_sb = cst.tile([1, C], F16)
            ks_sb = cst.tile([1, C], F16)
            bp_row = cst.tile([1, C], F16)
            nc.sync.dma_start(bv_sb[:], bv_d.ap().unsqueeze(0))
            nc.sync.dma_start(ks_sb[:], ks_d.ap().unsqueeze(0))
            nc.sync.dma_start(qs_sb[:], qs_d.ap().unsqueeze(0))
            nc.sync.dma_start(bp_row[:], bp_r_d.ap().unsqueeze(0))

            xT = in_pool.tile([128, KT, TN], F16, name="xTs")
            nc.sync.dma_start(xT[:], xT_d.ap().rearrange("(k p) t -> p k t", p=128))

            qT = [qt_pool.tile([128, TN], F16, tag=f"qT{m}", name=f"qT{m}") for m in range(MT)]
            kT = [kt_pool.tile([128, S], F16, tag=f"kT{m}", name=f"kT{m}") for m in range(MT)]
            vf = [vt_pool.tile([128, H, DA], F16, tag=f"vf{j}", name=f"vf{j}") for j in range(ST)]
            # wide accumulators: cols 0:512 even heads, 512:1024 odd heads
            acc = [acc_pool.tile([128, 2 * TN], F16, tag=f"acc{j}", name=f"acc{j}") for j in range(ST)]
            yT = [yt_pool.tile([128, TN], F16, tag=f"yT{k}", name=f"yT{k}") for k in range(KT)]

            # y_aug denominator column: 16.0 so 1/y_aug[64] = 1/(16 Z)
            for j in range(ST):
                nc.vector.memset(vf[j][:, :, D], 16.0)

            # ============ k projection + RMS (full S, wide N=1024) ============
            with (
                tc.tile_pool(name="pk_sq", bufs=3) as pk_sq,
                tc.tile_pool(name="pk_ps", bufs=2, space="PSUM") as pk_ps,
                tc.tile_pool(name="pk_ss", bufs=1, space="PSUM") as pk_ss,
                tc.tile_pool(name="pk_f", bufs=1, space="PSUM") as pk_f,
            ):
                ssum = [pk_ss.tile([1, 512], F32, tag=f"ssum{sh}", name=f"kss{sh}")
                        for sh in range(2)]
                for m in range(MT):
                    ps = pk_ps.tile([128, S], F32, tag="proj")
                    for sh in range(2):
                        for k in range(KT):
                            nc.tensor.matmul(
                                ps[:, sh * 512 : (sh + 1) * 512],
                                wk_sb[:, k, m * 128 : (m + 1) * 128],
                                eT[:, k, sh * 512 : (sh + 1) * 512],
                                start=(k == 0),
                                stop=(k == KT - 1),
                            )
                    nc.vector.tensor_scalar_add(kT[m][:], ps[:], bk_sb[:, m : m + 1])
                    sq = pk_sq.tile([128, S], F16, tag="sq")
                    nc.vector.tensor_mul(sq[:], kT[m][:], kT[m][:])
                    for sh in range(2):
                        nc.tensor.matmul(
                            ssum[sh][:], ones_col[:],
                            sq[:, sh * 512 : (sh + 1) * 512],
                            start=(m == 0), stop=(m == MT - 1),
                        )
                krr = pk_sq.tile([1, S], F16, tag="krr", name="krr")
                for sh in range(2):
                    rms = pk_sq.tile([1, 512], F32, tag="rms")
                    nc.scalar.activation(
                        rms[:], ssum[sh][:], AF.Sqrt, scale=1.0 / C, bias=eps_t[:]
                    )
                    with nc.allow_low_precision(reason="rms rsqrt broadcast"):
                        nc.vector.reciprocal(krr[:, sh * 512 : (sh + 1) * 512], rms[:])
                for m in range(MT):
                    fps = pk_f.tile([128, S], F32, tag="fps")
                    for sh in range(2):
                        nc.tensor.matmul(
                            fps[:, sh * 512 : (sh + 1) * 512],
                            ks_sb[:, m * 128 : (m + 1) * 128],
                            krr[:, sh * 512 : (sh + 1) * 512],
                            start=True, stop=True,
                        )
                    f1 = pk_sq.tile([128, S], F16, tag="f1")
                    nc.scalar.activation(f1[:], fps[:], AF.Copy, bias=1.0)
                    nc.vector.tensor_mul(kT[m][:], kT[m][:], f1[:])

            # v projection for one head half (heads jh*8 .. jh*8+7), all s
            def v_proj(pv_ps, wv_sb, jh, sts):
                for st in sts:
                    ps = pv_ps.tile([128, 512], F32, tag="proj")
                    for k in range(KT):
                        nc.tensor.matmul(
                            ps[:],
                            eT[:, k, st * 128 : (st + 1) * 128],
                            wv_sb[:, k, jh * 512 : (jh + 1) * 512],
                            start=(k == 0),
                            stop=False,
                        )
                    nc.tensor.matmul(
                        ps[:], ones_row[:], bv_sb[:, jh * 512 : (jh + 1) * 512],
                        start=False, stop=True,
                    )
                    nc.scalar.activation(
                        vf[st][:, jh * 8 : (jh + 1) * 8, 0:D],
                        ps[:].rearrange("p (h d) -> p h d", h=8),
                        AF.Copy,
                    )

            # ============ v projection heads 0-7 ============
            wv_sb = w_pool.tile([128, KT, C], F16, tag="W", name="wv_sb")
            with (
                tc.tile_pool(name="pv0_ps", bufs=2, space="PSUM") as pv0_ps,
            ):
                wide_load(wv_sb, wv_d, chunks=2)
                v_proj(pv0_ps, wv_sb, 0, range(ST))

            # ============ q projection + RMS (TN rows) ============
            with (
                tc.tile_pool(name="pq_sq", bufs=3) as pq_sq,
                tc.tile_pool(name="pq_ps", bufs=2, space="PSUM") as pq_ps,
                tc.tile_pool(name="pq_ss", bufs=1, space="PSUM") as pq_ss,
            ):
                wq_sb = w_pool.tile([128, KT, C], F16, tag="W", name="wq_sb")
                wide_load(wq_sb, wq_d, chunks=2)
                qss = pq_ss.tile([1, TN], F32, tag="qss", name="qss")
                for m in range(MT):
                    ps = pq_ps.tile([128, TN], F32, tag="proj")
                    for k in range(KT):
                        nc.tensor.matmul(
                            ps[:],
                            wq_sb[:, k, m * 128 : (m + 1) * 128],
                            xT[:, k, :],
                            start=(k == 0),
                            stop=(k == KT - 1),
                        )
                    nc.vector.tensor_scalar_add(qT[m][:], ps[:], bq_sb[:, m : m + 1])
                    sq = pq_sq.tile([128, TN], F16, tag="sq")
                    nc.vector.tensor_mul(sq[:], qT[m][:], qT[m][:])
                    nc.tensor.matmul(
                        qss[:], ones_col[:], sq[:],
                        start=(m == 0), stop=(m == MT - 1),
                    )
                qrms = pq_sq.tile([1, TN], F32, tag="qrms", name="qrms")
                nc.scalar.activation(
                    qrms[:], qss[:], AF.Sqrt, scale=1.0 / C, bias=eps_t[:]
                )
                qrr = pq_sq.tile([1, TN], F16, tag="qrr", name="qrr")
                with nc.allow_low_precision(reason="rms rsqrt broadcast"):
                    nc.vector.reciprocal(qrr[:], qrms[:])
                for m in range(MT):
                    fps = pq_ps.tile([128, TN], F32, tag="proj")
                    nc.tensor.matmul(
                        fps[:], qs_sb[:, m * 128 : (m + 1) * 128], qrr[:],
                        start=True, stop=True,
                    )
                    f1 = pq_sq.tile([128, TN], F16, tag="qf1")
                    if qf1_dve:
                        nc.vector.tensor_scalar_add(f1[:], fps[:], 1.0)
                    else:
                        nc.scalar.activation(f1[:], fps[:], AF.Copy, bias=1.0)
                    nc.vector.tensor_mul(qT[m][:], qT[m][:], f1[:])

            # ---- attention (pairs 0-3 interleave v-proj heads 8-15) ----
            if True:
                with (
                    tc.tile_pool(name="p3_e", bufs=2) as p3_e,
                    tc.tile_pool(name="p3_rb", bufs=2) as p3_rb,
                    tc.tile_pool(name="p3_t", bufs=3) as p3_t,
                    tc.tile_pool(name="pv_ps", bufs=2, space="PSUM") as pv_ps,
                    tc.tile_pool(name="p3_sc", bufs=2, space="PSUM") as p3_sc,
                    tc.tile_pool(name="p3_y", bufs=2, space="PSUM") as p3_y,
                ):
                    wp_sb = w_pool.tile([128, KT, C], F16, tag="W", name="wp_sb")
                    wide_load(wp_sb, wp_d)

                    def attn_pair(p):
                        mt = p
                        heads = [(2 * p, 0), (2 * p + 1, 64)]
                        exps = []
                        for j in range(ST):
                            sc = p3_sc.tile([128, 2 * TN], F32, tag="sc")
                            for h, base in heads:
                                nc.tensor.matmul(
                                    sc[:, base * 8 : base * 8 + TN],
                                    kT[mt][base : base + 64, j * 128 : (j + 1) * 128],
                                    qT[mt][base : base + 64, :],
                                    start=True,
                                    stop=True,
                                )
                            ex = p3_e.tile([128, 2 * TN], F16, tag=f"exp{j}")
                            nc.scalar.activation(
                                ex[:], sc[:], AF.Exp, scale=float(D) ** -0.5,
                                bias=ebias_t[:],
                            )
                            exps.append(ex)
                        recip = p3_rb.tile([1, 2 * TN], F16, tag="recip")
                        rb_sb = p3_rb.tile([128, 2 * TN], F16, tag="rbsb")
                        y_pss = []
                        for h, base in heads:
                            y_ps = p3_y.tile([DA, TN], F32, tag="y")
                            y_pss.append(y_ps)
                            for j in range(ST):
                                nc.tensor.matmul(
                                    y_ps[:],
                                    vf[j][:, h, :],
                                    exps[j][:, base * 8 : base * 8 + TN],
                                    start=(j == 0),
                                    stop=(j == ST - 1),
                                )
                            # per-head rb = 1/(16 Z) broadcast: head A's chain
                            # completes while head B's y matmuls run
                            with nc.allow_low_precision(reason="softmax 1/Z bcast"):
                                nc.vector.reciprocal(
                                    recip[:, base * 8 : base * 8 + TN], y_ps[64:65, :]
                                )
                            rb_ps = p3_sc.tile([128, 2 * TN], F32, tag="sc")
                            nc.tensor.matmul(
                                rb_ps[:, 0:TN],
                                ones_row[:],
                                recip[:, base * 8 : base * 8 + TN],
                                start=True, stop=True,
                            )
                            nc.scalar.activation(
                                rb_sb[:, base * 8 : base * 8 + TN],
                                rb_ps[:, 0:TN], AF.Copy,
                            )
                        # yT rows = 16 * y_aug * rb
                        for (h, base), y_ps in zip(heads, y_pss):
                            (nc.gpsimd if stt_pool else nc.vector).scalar_tensor_tensor(
                                yT[mt][base : base + 64, :],
                                y_ps[0:64, :],
                                16.0,
                                rb_sb[0:64, base * 8 : base * 8 + TN],
                                ALU.mult,
                                ALU.mult,
                            )
                        # attn_mean: acc[j] += exp[j] * rb (wide, both heads)
                        for j in range(ST):
                            eng = nc.vector if j < dve_j else nc.gpsimd
                            if p == 0:
                                eng.tensor_mul(acc[j][:], exps[j][:], rb_sb[:])
                            else:
                                t = p3_t.tile([128, 2 * TN], F16, tag=f"t{j % 3}")
                                eng.tensor_mul(t[:], exps[j][:], rb_sb[:])
                                eng.tensor_add(acc[j][:], acc[j][:], t[:])

                    for p in range(4):
                        attn_pair(p)
                        v_proj(pv_ps, wv_sb, 1, range(2 * p, 2 * p + 2))
                    for p in range(4, 8):
                        attn_pair(p)

                    # output projection (only needs yT; emitted before the
                    # attn_mean epilogue so PE overlaps the elementwise tail;
                    # bias via K=1 ones matmul + ACT copy keeps DVE free)
                    for m in range(MT):
                        ps = pv_ps.tile([128, TN], F32, tag="proj")
                        for k in range(KT):
                            nc.tensor.matmul(
                                ps[:],
                                wp_sb[:, k, m * 128 : (m + 1) * 128],
                                yT[k][:],
                                start=(k == 0),
                                stop=False,
                            )
                        nc.tensor.matmul(
                            ps[:],
                            bp_row[:, m * 128 : (m + 1) * 128],
                            ones_wrow[:],
                            start=False,
                            stop=True,
                        )
                        yo = p3_t.tile([128, TN], F16, tag=f"yo{m % 2}")
                        nc.scalar.activation(yo[:], ps[:], AF.Copy)
                        nc.sync.dma_start(yT_o.ap()[m * 128 : (m + 1) * 128, :], yo[:])

                    for j in range(ST):
                        amo = p3_t.tile([128, TN], F16, tag=f"amo{j % 2}")
                        eng = nc.gpsimd if j % 2 else nc.vector
                        eng.tensor_add(amo[:], acc[j][:, 0:TN], acc[j][:, TN:])
                        nc.sync.dma_start(
                            am_o.ap()[j * 128 : (j + 1) * 128, :], amo[:]
                        )

    nc.compile()
    return nc


_NC_CACHE = None


def _get_nc():
    global _NC_CACHE
    if _NC_CACHE is None:
        _NC_CACHE = build()
    return _NC_CACHE


def make_in_maps(x, encoder_output, Wq, bq, Wk, bk, Wv, bv, q_scale, k_scale,
                 Wp, bp):
    f16 = np.float16
    x = np.asarray(x, np.float32)
    enc = np.asarray(encoder_output, np.float32)
    Wq = np.ascontiguousarray(np.asarray(Wq, f16))
    Wk = np.ascontiguousarray(np.asarray(Wk, f16))
    Wv = np.ascontiguousarray(np.asarray(Wv, f16))
    Wp = np.ascontiguousarray(np.asarray(Wp, f16))
    bq_t = np.ascontiguousarray(np.asarray(bq, np.float32).reshape(MT, 128).T)
    bk_t = np.ascontiguousarray(np.asarray(bk, np.float32).reshape(MT, 128).T)
    bp_r = np.ascontiguousarray(np.asarray(bp, f16))
    bv = np.ascontiguousarray(np.asarray(bv, f16))
    qs = np.ascontiguousarray(np.asarray(q_scale, f16))
    ks = np.ascontiguousarray(np.asarray(k_scale, f16))

    in_maps = []
    for c in range(8):
        b, hf = c // 2, c % 2
        xT = np.ascontiguousarray(x[b, hf * TN : (hf + 1) * TN, :].T.astype(f16))
        encT = np.ascontiguousarray(enc[b].T.astype(f16))
        in_maps.append(
            dict(xT=xT, encT=encT, wq=Wq, wk=Wk, wv=Wv, wp=Wp,
                 bq=bq_t, bk=bk_t, bp_r=bp_r, bv=bv, qs=qs, ks=ks)
        )
    return in_maps


def kernel(x, encoder_output, Wq, bq, Wk, bk, Wv, bv, q_scale, k_scale, Wp, bp,
           _trace=False):
    in_maps = make_in_maps(x, encoder_output, Wq, bq, Wk, bk, Wv, bv, q_scale,
                           k_scale, Wp, bp)
    nc = _get_nc()
    res = run_bass_kernel_spmd(nc, in_maps, core_ids=list(range(8)), trace=_trace)

    y = np.empty((B, T, C), np.float32)
    amean = np.empty((B, T, S), np.float32)
    for c in range(8):
        b, hf = c // 2, c % 2
        r = res.results[c]
        y[b, hf * TN : (hf + 1) * TN, :] = r["youtT"].T.astype(np.float32)
        amean[b, hf * TN : (hf + 1) * TN, :] = r["ameanT"].T.astype(np.float32)
    if _trace:
        kernel.last_exec_time_ns = res.exec_time_ns
        kernel.last_results = res
    return y, amean
